# revision 2
# baseline (speedup 1.0000x reference)
"""Cluster-based contrastive loss on 8 Trainium2 NeuronCores — v2.

Fully-local design: NO collectives.  Every core builds the complete
normalized pair table [128, 12800] itself (topk + gather + normalize +
transpose for all 50 clusters), then sweeps only its OWN 7 cluster
slots (14 row blocks of 128) against the full table.  Work assignment
is encoded entirely in per-core INPUTS (probw column permutation puts
the core's own clusters in slots 0..6; wfin masks duplicate slots), so
the emitted program is identical on all cores — no partition_id, no
cross-core synchronization, no launch-skew exposure.

Build pipeline is emitted in 4 slot-groups (13/13/13/11) so topk of
group g+1 overlaps scatter/gather/transpose of group g, and the sweep
overlaps the tail of the build via subtile dependencies.

The sim sweep: per row block, 25 matmuls of 512 cols into [128,1536]
PSUM tiles (3 banks x 2 bufs) + 9 exp-activations with accum_out.
Own-cluster/positive sums are extracted from the sweep's own tile
(the e values for cols [256s, 256s+256) live in sweep tile 0 or 1 at a
compile-time offset).  All data movement copies run on DVE, keeping the
Activation engine for exp/ln only.
"""

import sys

sys.path.insert(0, "/opt/trn_rl_repo")

import numpy as np

import concourse.bacc as bacc
import concourse.bass as bass
import concourse.mybir as mybir
from concourse import tile
from concourse.bass_utils import run_bass_kernel_spmd

F32 = mybir.dt.float32
BF16 = mybir.dt.bfloat16
I16 = mybir.dt.int16
U16 = mybir.dt.uint16
AF = mybir.ActivationFunctionType
ALU = mybir.AluOpType

B = 16384
D = 128
C = 50
K = 128
TEMP = 0.5
N_CORES = 8
RSLOT = 7                      # row slots swept per core (own + pad)
NRB = 2 * RSLOT                # 14 swept row blocks
TBL = C * 2 * K                # 12800 table columns, no padding
CCNT = [7, 7, 6, 6, 6, 6, 6, 6]
CBASE = [0, 7, 14, 20, 26, 32, 38, 44]
QUANTILE = 1.0 - 127.5 / (B - 1)
GROUPS = [(0, 8), (8, 22), (22, 36), (36, 50)]
ST = 1536                      # sweep PSUM tile width (3 banks)
MM_CHUNK = 512                 # matmul out cap: one PSUM bank
# sweep tiles per row block: 8 x 1536 + 1 x 512
ST_SIZES = [ST] * 8 + [512]
N_ST = len(ST_SIZES)
# packed big-constant layout (columns)
BC_LEXCL = 0
BC_IDENT = 128
BC_RANKB = 256                 # [1,400] replicated across partitions
BC_POFF = 656
BC_REP16 = 657                 # [16,128] block in partitions 0..15
BC_COLS = 785

_CACHE = {}


def _host_constants():
    bigc = np.zeros((128, BC_COLS), dtype=np.float32)
    bigc[:, BC_LEXCL : BC_LEXCL + 128] = (
        np.arange(128)[:, None] < np.arange(128)[None, :]
    )
    bigc[:, BC_IDENT : BC_IDENT + 128] = np.eye(128)
    # group-local rank base: rb[8s+t] = 128*(s - gstart) + t + 1
    rb = np.zeros(8 * C, dtype=np.float32)
    for g0, g1 in GROUPS:
        for s in range(g0, g1):
            for t in range(8):
                rb[8 * s + t] = 128 * (s - g0) + t + 1
    bigc[:, BC_RANKB : BC_RANKB + 8 * C] = rb[None, :]
    bigc[:, BC_POFF] = 128.0 * np.arange(128)
    bigc[0:16, BC_REP16 : BC_REP16 + 128] = (
        np.arange(128)[None, :] % 16 == np.arange(16)[:, None]
    )
    return bigc


def _build_program(repeats=1, variant="full"):
    nc = bacc.Bacc(
        "TRN2", target_bir_lowering=False, debug=False, num_devices=N_CORES
    )

    probw = nc.dram_tensor("probw", [128, C * 128], F32, kind="ExternalInput")
    z = nc.dram_tensor("z", [2 * B, D], F32, kind="ExternalInput")
    bigc = nc.dram_tensor("bigc", [128, BC_COLS], F32, kind="ExternalInput")
    wfin = nc.dram_tensor("wfin", [1, NRB], F32, kind="ExternalInput")
    out0 = nc.dram_tensor("partial0", [1, 1], F32, kind="ExternalOutput")

    with tile.TileContext(nc) as tc:
        for r in range(repeats):
            _emit(nc, tc, probw, z, bigc, wfin, out0, rep=r, variant=variant)

    nc.compile()
    return nc


def _emit(nc, tc, probw, z, bigc, wfin, out, rep=0, variant="full"):
    from contextlib import ExitStack

    R = f"r{rep}_"
    ctx = ExitStack()
    with ctx:
        const = ctx.enter_context(tc.tile_pool(name=R + "const", bufs=1))
        main = ctx.enter_context(tc.tile_pool(name=R + "main", bufs=1))
        scr = ctx.enter_context(tc.tile_pool(name=R + "scr", bufs=2))
        gscr = ctx.enter_context(tc.tile_pool(name=R + "gscr", bufs=2))
        escr = ctx.enter_context(tc.tile_pool(name=R + "escr", bufs=4))
        psum_sim = ctx.enter_context(
            tc.tile_pool(name=R + "psum_sim", bufs=2, space="PSUM")
        )
        build_ctx = ExitStack()
        probp = build_ctx.enter_context(tc.tile_pool(name=R + "probp", bufs=1))
        psum_b = build_ctx.enter_context(
            tc.tile_pool(name=R + "psum_b", bufs=2, space="PSUM")
        )
        dram = ctx.enter_context(tc.tile_pool(name=R + "dram", bufs=1, space="DRAM"))

        # ---- constants (2 DMAs) ----------------------------------------
        bigc_sb = const.tile([128, BC_COLS], F32, tag="bigc")
        wfin_sb = const.tile([1, NRB], F32, tag="wfin")
        nc.sync.dma_start(bigc_sb[:], bigc.ap())
        nc.sync.dma_start(wfin_sb[:], wfin.ap())
        lexcl_sb = bigc_sb[:, BC_LEXCL : BC_LEXCL + 128]
        ident_sb = bigc_sb[:, BC_IDENT : BC_IDENT + 128]
        rankb_sb = bigc_sb[:, BC_RANKB : BC_RANKB + 8 * C]
        poff_sb = bigc_sb[:, BC_POFF : BC_POFF + 1]
        rep16_sb = bigc_sb[0:16, BC_REP16 : BC_REP16 + 128]
        ones_p = const.tile([128, 1], F32, tag="ones_p")
        ones_r = const.tile([1, 128], F32, tag="ones_r")
        nc.vector.memset(ones_p[:], 1.0)
        nc.vector.memset(ones_r[:], 1.0)

        # ---- build: per-group topk -> indices -> gather ----------------
        prob_sb = probp.tile([128, C, 128], F32, tag="prob")
        if "mini" in variant:
            nc.sync.dma_start(prob_sb[:], probw.ap())
            mini_ps = psum_b.tile([128, 128], F32, tag="sm128")
            nc.tensor.matmul(mini_ps[:], lexcl_sb, prob_sb[:, 0, :])
            mini = main.tile([1, 1], F32, tag="mini")
            nc.vector.tensor_reduce(
                mini[:], mini_ps[0:1, :], axis=mybir.AxisListType.X, op=ALU.add
            )
            nc.sync.dma_start(out[:], mini[:])
            build_ctx.close()
            return

        taus = main.tile([1, 2 * C], F32, tag="taus")
        vals3 = main.tile([128, C, 8], F32, tag="vals3")
        idxs3 = main.tile([128, C, 8], U16, tag="idxs3")
        rank16 = main.tile([128, 8 * C], I16, tag="rank16")
        cand16 = main.tile([128, 8 * C], I16, tag="cand16")
        evr16 = main.tile([128, C * 128], I16, tag="evr16")
        idx_dram = dram.tile([1, C * 128], F32, name="idx_dram")
        idxs_i16 = main.tile([128, 2, 8 * C], I16, tag="idxs")
        g = main.tile([128, 2, C, 128], F32, tag="g")

        def _stop(src_ap):
            stp = main.tile([1, 1], F32, tag="stop")
            nc.vector.tensor_reduce(
                stp[:], src_ap, axis=mybir.AxisListType.X, op=ALU.add
            )
            nc.sync.dma_start(out[:], stp[:])
            build_ctx.close()

        def emit_build_AD(gi):
            g0, g1 = GROUPS[gi]
            gsz = g1 - g0
            nc.sync.dma_start(
                prob_sb[:, g0:g1, :], probw.ap()[:, 128 * g0 : 128 * g1]
            )
            # -- stage A: exact thresholds (gpsimd) ----------------------
            for s in range(g0, g1):
                nc.gpsimd.kth_largest(
                    taus[0:1, 2 * s : 2 * s + 2],
                    prob_sb[:, s, :],
                    n_per_lane=128,
                    k=K + 2,
                    quantile=QUANTILE,
                )
            taub_ps = psum_b.tile([128, 512], F32, tag="bsm")
            nc.tensor.matmul(
                taub_ps[:, 0:gsz],
                ones_r[:],
                taus[0:1, 2 * g0 + 1 : 2 * g1 : 2],
            )
            # -- stage B: per-partition top-8 + ranking ------------------
            for s in range(g0, g1):
                nc.vector.max(vals3[:, s, :], prob_sb[:, s, :])
                nc.vector.max_index(idxs3[:, s, :], vals3[:, s, :], prob_sb[:, s, :])
            valid3 = scr.tile([128, 15, 8], F32, tag="valid3")
            nc.vector.tensor_tensor(
                valid3[:, 0:gsz, :],
                vals3[:, g0:g1, :],
                taub_ps[:, 0:gsz]
                .rearrange("p (c o) -> p c o", o=1)
                .to_broadcast([128, gsz, 8]),
                op=ALU.is_gt,
            )
            rowcnt = scr.tile([128, 15], F32, tag="rowcnt")
            nc.vector.tensor_reduce(
                rowcnt[:, 0:gsz], valid3[:, 0:gsz, :],
                axis=mybir.AxisListType.X, op=ALU.add,
            )
            rowoff_ps = psum_b.tile([128, 512], F32, tag="bsm")
            nc.tensor.matmul(rowoff_ps[:, 0:gsz], lexcl_sb, rowcnt[:, 0:gsz])
            ranks = scr.tile([128, 15, 8], F32, tag="ranks")
            nc.vector.tensor_tensor(
                ranks[:, 0:gsz, :],
                rowoff_ps[:, 0:gsz]
                .rearrange("p (c o) -> p c o", o=1)
                .to_broadcast([128, gsz, 8]),
                rankb_sb[:, 8 * g0 : 8 * g1].rearrange("p (c t) -> p c t", t=8),
                op=ALU.add,
            )
            nc.vector.tensor_tensor(
                ranks[:, 0:gsz, :], ranks[:, 0:gsz, :], valid3[:, 0:gsz, :],
                op=ALU.mult,
            )
            nc.vector.tensor_scalar_add(ranks[:, 0:gsz, :], ranks[:, 0:gsz, :], -1.0)
            nc.vector.tensor_copy(
                rank16[:, 8 * g0 : 8 * g1],
                ranks[:, 0:gsz, :].rearrange("p c t -> p (c t)"),
            )
            gidxf = scr.tile([128, 15 * 8], F32, tag="gidxf")
            nc.vector.tensor_scalar(
                gidxf[:, 0 : 8 * gsz],
                idxs3[:, g0:g1, :].rearrange("p c t -> p (c t)"),
                poff_sb, None, op0=ALU.add,
            )
            nc.vector.tensor_copy(cand16[:, 8 * g0 : 8 * g1], gidxf[:, 0 : 8 * gsz])
            # -- compact to batch-index list (gpsimd scatter) ------------
            nc.gpsimd.local_scatter(
                evr16[:, 128 * g0 : 128 * g1],
                cand16[:, 8 * g0 : 8 * g1],
                rank16[:, 8 * g0 : 8 * g1],
                channels=128, num_elems=128 * gsz, num_idxs=8 * gsz,
            )
            evrf = gscr.tile([128, 15 * 128], F32, tag="evrf")
            nc.vector.tensor_copy(
                evrf[:, 0 : 128 * gsz], evr16[:, 128 * g0 : 128 * g1]
            )
            # cross-partition collapse, 512-col chunks, psum -> SBUF row
            allidx = main.tile([1, C * 128], F32, tag="allidx")
            nchunk = (128 * gsz + MM_CHUNK - 1) // MM_CHUNK
            for ci in range(nchunk):
                c0 = MM_CHUNK * ci
                c1 = min(MM_CHUNK * (ci + 1), 128 * gsz)
                aps = psum_b.tile([128, 512], F32, tag="bsm")
                nc.tensor.matmul(aps[0:1, 0 : c1 - c0], ones_p[:], evrf[:, c0:c1])
                nc.vector.tensor_copy(
                    allidx[:, 128 * g0 + c0 : 128 * g0 + c1],
                    aps[0:1, 0 : c1 - c0],
                )
            # -- wrap into 16 partitions (DRAM roundtrip) ---------------
            nc.sync.dma_start(
                idx_dram[:, 128 * g0 : 128 * g1],
                allidx[:, 128 * g0 : 128 * g1],
            )
            wrapped = main.tile([16, 8 * C], F32, tag="wrapped")
            nc.sync.dma_start(
                wrapped[:, 8 * g0 : 8 * g1],
                idx_dram[:, 128 * g0 : 128 * g1].rearrange(
                    "p (s m) -> (p m) s", m=16
                ),
            )
            widx_ps = psum_b.tile([128, 512], F32, tag="bsm")
            nc.tensor.matmul(
                widx_ps[:, 0 : 8 * gsz], rep16_sb, wrapped[:, 8 * g0 : 8 * g1]
            )
            nc.vector.tensor_copy(
                idxs_i16[:, 0, 8 * g0 : 8 * g1], widx_ps[:, 0 : 8 * gsz]
            )
            nc.vector.tensor_scalar_add(
                idxs_i16[:, 1, 8 * g0 : 8 * g1], widx_ps[:, 0 : 8 * gsz], float(B)
            )
            # -- stage D: gathers (<=1024 idxs each) ---------------------
            for h in range(2):
                splits = [(i, min(i + 8, gsz)) for i in range(0, gsz, 8)]
                for s0, s1 in splits:
                    nidx = 128 * (s1 - s0)
                    nc.gpsimd.dma_gather(
                        g[:, h, g0 + s0 : g0 + s1, :],
                        z.ap(),
                        idxs_i16[:, h, 8 * (g0 + s0) : 8 * (g0 + s1)],
                        num_idxs=nidx, num_idxs_reg=nidx, elem_size=D,
                    )

        # ---- stage E: normalize + fused transpose into bf16 table ------
        # one table tile per group (512-aligned) so the sweep's reads only
        # depend on the producing group's writes
        flats = []
        for gi, (g0, g1) in enumerate(GROUPS):
            ft = main.tile(
                [128, 256 * (g1 - g0)], BF16, tag=f"flatT{gi}",
                name=f"{R}flatT{gi}",
            )
            flats.append(ft)

        def tbl(c0, c1):
            """AP for global table cols [c0, c1) — must lie in one group."""
            for (g0, g1), ft in zip(GROUPS, flats):
                if c0 >= 256 * g0 and c1 <= 256 * g1:
                    return ft[:, c0 - 256 * g0 : c1 - 256 * g0]
            raise AssertionError((c0, c1))

        rnw = main.tile([128, C, 2], F32, tag="rnw")

        def emit_build_E(gi):
            g0, g1 = GROUPS[gi]
            gsz = g1 - g0
            sqs = scr.tile([128, 15, 2], F32, tag="sqs")
            for h in range(2):
                sqg = gscr.tile([128, 15, 128], F32, tag="sqg")
                nc.vector.tensor_tensor(
                    sqg[:, 0:gsz, :], g[:, h, g0:g1, :], g[:, h, g0:g1, :],
                    op=ALU.mult,
                )
                nc.vector.tensor_reduce(
                    sqs[:, 0:gsz, h], sqg[:, 0:gsz, :],
                    axis=mybir.AxisListType.X, op=ALU.add,
                )
            lnv = scr.tile([128, 15, 2], F32, tag="lnv")
            nc.scalar.activation(lnv[:, 0:gsz, :], sqs[:, 0:gsz, :], AF.Ln)
            nc.scalar.activation(
                rnw[:, g0:g1, :], lnv[:, 0:gsz, :], AF.Exp, scale=-0.5
            )
            # diag(rnw_block) per 2-cluster pack (4 blocks, h interleaved);
            # one f32 matmul per block fuses normalize+transpose; the pack
            # lands contiguously: flatT[:, 256c .. 256c+512)
            for p0 in range(0, gsz, 2):
                npk = min(2, gsz - p0)
                diag4 = gscr.tile([128, 2, 2, 128], F32, tag="diag4")
                nc.vector.tensor_tensor(
                    diag4[:, 0:npk, :, :],
                    ident_sb.rearrange("p (o u f) -> p o u f", o=1, u=1)
                    .to_broadcast([128, npk, 2, 128]),
                    rnw[:, g0 + p0 : g0 + p0 + npk, :]
                    .rearrange("p c (h o) -> p c h o", o=1)
                    .to_broadcast([128, npk, 2, 128]),
                    op=ALU.mult,
                )
                tp_ps = psum_b.tile([128, 512], F32, tag="bsm")
                for j in range(npk):
                    for h in range(2):
                        nc.tensor.matmul(
                            tp_ps[:, 256 * j + 128 * h : 256 * j + 128 * h + 128],
                            g[:, h, g0 + p0 + j, :],
                            diag4[:, j, h, :],
                        )
                nc.vector.tensor_copy(
                    flats[gi][:, 256 * p0 : 256 * p0 + 256 * npk],
                    tp_ps[:, 0 : 256 * npk],
                )

        partials = main.tile([128, NRB, N_ST], F32, tag="partials")
        eown = main.tile([128, NRB, 256], BF16, tag="eown")

        def emit_sweep_st(st):
            col = ST * st
            stw = ST_SIZES[st]
            for n in range(NRB):
                s, h = n % RSLOT, n // RSLOT
                lhsT = tbl(256 * s + 128 * h, 256 * s + 128 * h + 128)
                sim_ps = psum_sim.tile([128, ST], F32, tag="sim")
                for q0 in range(0, stw, MM_CHUNK):
                    nc.tensor.matmul(
                        sim_ps[:, q0 : q0 + MM_CHUNK],
                        lhsT,
                        tbl(col + q0, col + q0 + MM_CHUNK),
                    )
                e_sb = escr.tile([128, ST], BF16, tag="e")
                nc.scalar.activation(
                    e_sb[:, 0:stw],
                    sim_ps[:, 0:stw],
                    AF.Exp,
                    scale=1.0 / TEMP,
                    accum_out=partials[:, n, st : st + 1],
                )
                # own-cluster block: cols [256s, 256s+256) sit entirely in
                # tile 0 (s<6) or at the head of tile 1 (s==6)
                own_st, own_off = (0, 256 * s) if s < 6 else (1, 0)
                if st == own_st:
                    nc.vector.tensor_copy(
                        eown[:, n, :], e_sb[:, own_off : own_off + 256]
                    )

        # interleave emission: per-engine queues are in-order, so sweep
        # tiles must be emitted as soon as their producing groups are
        own_t = main.tile([128, NRB], F32, tag="own_t")
        pos_t = main.tile([128, NRB], F32, tag="pos_t")
        lnp = scr.tile([128, NRB], F32, tag="lnp")

        def emit_own_reduce():
            nc.vector.tensor_reduce(
                own_t[:], eown[:], axis=mybir.AxisListType.X, op=ALU.add
            )
            nc.vector.tensor_reduce(
                pos_t[:], eown[:, :, 0:128], axis=mybir.AxisListType.X, op=ALU.add
            )
            nc.scalar.activation(lnp[:], pos_t[:], AF.Ln)

        SWEEP_AFTER = [[0], [1, 2], [3, 4, 5], [6, 7, 8]]
        for gi in range(len(GROUPS)):
            emit_build_AD(gi)
            if "stopD" in variant:
                continue
            emit_build_E(gi)
            if "stopE" in variant or "nosweep" in variant:
                continue
            for st in SWEEP_AFTER[gi]:
                emit_sweep_st(st)
            if gi == 1:
                emit_own_reduce()
        if "stopD" in variant:
            _stop(g[0:1, 0, 0, :])
            return
        if "stopE" in variant or "nosweep" in variant:
            ef = scr.tile([1, 2048], F32, tag="ef")
            nc.vector.tensor_copy(ef[:], flats[0][0:1, :])
            _stop(ef[0:1, :])
            return
        # ---- reduce to one scalar --------------------------------------
        totals = main.tile([128, NRB], F32, tag="totals")
        nc.vector.tensor_reduce(
            totals[:], partials[:], axis=mybir.AxisListType.X, op=ALU.add
        )
        neg = scr.tile([128, NRB], F32, tag="neg")
        nc.vector.tensor_sub(neg[:], totals[:], own_t[:])
        lnn = scr.tile([128, NRB], F32, tag="lnn")
        nc.scalar.activation(lnn[:], neg[:], AF.Ln)
        lrows = main.tile([128, NRB], F32, tag="lrows")
        nc.vector.tensor_sub(lrows[:], lnn[:], lnp[:])
        fin_ps = psum_sim.tile([1, NRB], F32, tag="sim")
        nc.tensor.matmul(fin_ps[:], ones_p[:], lrows[:])
        fin_sb = main.tile([1, NRB], F32, tag="fin_sb")
        nc.vector.tensor_tensor(fin_sb[:], fin_ps[:], wfin_sb[:], op=ALU.mult)
        out_sb = main.tile([1, 1], F32, tag="out_sb")
        nc.vector.tensor_reduce(
            out_sb[:], fin_sb[:], axis=mybir.AxisListType.X, op=ALU.add
        )
        nc.vector.tensor_scalar_mul(out_sb[:], out_sb[:], 1.0 / (2 * K * C))
        nc.sync.dma_start(out[:], out_sb[:])
        build_ctx.close()


def _per_core_inputs(prob, z_i, z_j):
    bigc = _host_constants()
    zcat = np.ascontiguousarray(np.concatenate([z_i, z_j], axis=0))
    maps = []
    for k in range(N_CORES):
        cols = [(CBASE[k] + j) % C for j in range(C)]
        pw = np.ascontiguousarray(
            prob[:, cols].T.reshape(C, 128, 128).transpose(1, 0, 2)
            .reshape(128, C * 128)
        )
        w = np.array(
            [1.0 if s < CCNT[k] else 0.0 for s in range(RSLOT)], dtype=np.float32
        )
        wn = np.concatenate([w, w])  # n = s + 7h block order
        maps.append({
            "probw": pw,
            "z": zcat,
            "bigc": bigc,
            "wfin": wn[None, :].astype(np.float32).copy(),
        })
    return maps


def kernel(prob, z_i, z_j):
    if "nc" not in _CACHE:
        _CACHE["nc"] = _build_program()
    nc = _CACHE["nc"]
    in_maps = _per_core_inputs(
        np.asarray(prob, dtype=np.float32),
        np.asarray(z_i, dtype=np.float32),
        np.asarray(z_j, dtype=np.float32),
    )
    res = run_bass_kernel_spmd(nc, in_maps, list(range(N_CORES)))
    total = np.float32(0.0)
    for r in res.results:
        total += r["partial0"][0, 0]
    return np.asarray(total, dtype=np.float32)


# revision 5
# speedup vs baseline: 1.0765x; 1.0765x over previous
"""Cluster-based contrastive loss on 8 Trainium2 NeuronCores — v2.

Fully-local design: NO collectives.  Every core builds the complete
normalized pair table [128, 12800] itself (topk + gather + normalize +
transpose for all 50 clusters), then sweeps only its OWN 7 cluster
slots (14 row blocks of 128) against the full table.  Work assignment
is encoded entirely in per-core INPUTS (probw column permutation puts
the core's own clusters in slots 0..6; wfin masks duplicate slots), so
the emitted program is identical on all cores — no partition_id, no
cross-core synchronization, no launch-skew exposure.

Build pipeline is emitted in 4 slot-groups (13/13/13/11) so topk of
group g+1 overlaps scatter/gather/transpose of group g, and the sweep
overlaps the tail of the build via subtile dependencies.

The sim sweep: per row block, 25 matmuls of 512 cols into [128,1536]
PSUM tiles (3 banks x 2 bufs) + 9 exp-activations with accum_out.
Own-cluster/positive sums are extracted from the sweep's own tile
(the e values for cols [256s, 256s+256) live in sweep tile 0 or 1 at a
compile-time offset).  All data movement copies run on DVE, keeping the
Activation engine for exp/ln only.
"""

import sys

sys.path.insert(0, "/opt/trn_rl_repo")

import numpy as np

import concourse.bacc as bacc
import concourse.bass as bass
import concourse.mybir as mybir
from concourse import tile
from concourse.bass_utils import run_bass_kernel_spmd

F32 = mybir.dt.float32
BF16 = mybir.dt.bfloat16
I16 = mybir.dt.int16
U16 = mybir.dt.uint16
AF = mybir.ActivationFunctionType
ALU = mybir.AluOpType

B = 16384
D = 128
C = 50
K = 128
TEMP = 0.5
N_CORES = 8
RSLOT = 7                      # row slots swept per core (own + pad)
# 13 swept row blocks: slots 0..5 x both halves + slot 6 sub-block 0.
# Which z-half each sub-block holds is input-controlled (oofs), so the
# slot-6 sub-block can be either half of a "split" cluster.
SWEEP_SH = [(s, h) for s in range(6) for h in range(2)] + [(6, 0)]
NRB = len(SWEEP_SH)            # 13
TBL = C * 2 * K                # 12800 table columns, no padding
CCNT = [7, 7, 6, 6, 6, 6, 6, 6]
CBASE = [0, 7, 14, 20, 26, 32, 38, 44]
QUANTILE = 1.0 - 127.5 / (B - 1)
QUANT_CAND = 1.0 - 127.5 / (8 * 128 - 1)
GROUPS = [(0, 8), (8, 22), (22, 36), (36, 50)]
ST = 1536                      # sweep PSUM tile width (3 banks)
MM_CHUNK = 512                 # matmul out cap: one PSUM bank
# sweep tiles per row block: 8 x 1536 + 1 x 512
ST_SIZES = [ST] * 8 + [512]
N_ST = len(ST_SIZES)
# packed big-constant layout (columns)
BC_LEXCL = 0
BC_IDENT = 128
BC_RANKB = 256                 # [1,400] replicated across partitions
BC_POFF = 656
BC_REP16 = 657                 # [16,128] block in partitions 0..15
BC_OOFS = 785                  # [1,2*400] per-(h,slot) gather offset 0/B
BC_PSEL = 1585                 # [1,2*NRB] pos-half selector per swept block
BC_COLS = 1611

_CACHE = {}


def _host_constants():
    bigc = np.zeros((128, BC_COLS), dtype=np.float32)
    bigc[:, BC_LEXCL : BC_LEXCL + 128] = (
        np.arange(128)[:, None] < np.arange(128)[None, :]
    )
    bigc[:, BC_IDENT : BC_IDENT + 128] = np.eye(128)
    # group-local rank base: rb[8s+t] = 128*(s - gstart) + t + 1
    rb = np.zeros(8 * C, dtype=np.float32)
    for g0, g1 in GROUPS:
        for s in range(g0, g1):
            for t in range(8):
                rb[8 * s + t] = 128 * (s - g0) + t + 1
    bigc[:, BC_RANKB : BC_RANKB + 8 * C] = rb[None, :]
    bigc[:, BC_POFF] = 128.0 * np.arange(128)
    bigc[0:16, BC_REP16 : BC_REP16 + 128] = (
        np.arange(128)[None, :] % 16 == np.arange(16)[:, None]
    )
    return bigc


def _build_program(repeats=1, variant="full"):
    nc = bacc.Bacc(
        "TRN2", target_bir_lowering=False, debug=False, num_devices=N_CORES,
    )

    probw = nc.dram_tensor("probw", [128, C * 128], F32, kind="ExternalInput")
    z = nc.dram_tensor("z", [2 * B, D], F32, kind="ExternalInput")
    bigc = nc.dram_tensor("bigc", [128, BC_COLS], F32, kind="ExternalInput")
    wfin = nc.dram_tensor("wfin", [1, NRB], F32, kind="ExternalInput")
    out0 = nc.dram_tensor("partial0", [1, 1], F32, kind="ExternalOutput")

    with tile.TileContext(nc) as tc:
        for r in range(repeats):
            _emit(nc, tc, probw, z, bigc, wfin, out0, rep=r, variant=variant)

    nc.compile()
    return nc


def _emit(nc, tc, probw, z, bigc, wfin, out, rep=0, variant="full"):
    from contextlib import ExitStack

    R = f"r{rep}_"
    ctx = ExitStack()
    with ctx:
        const = ctx.enter_context(tc.tile_pool(name=R + "const", bufs=1))
        main = ctx.enter_context(tc.tile_pool(name=R + "main", bufs=1))
        scr = ctx.enter_context(tc.tile_pool(name=R + "scr", bufs=2))
        gscr = ctx.enter_context(tc.tile_pool(name=R + "gscr", bufs=2))
        escr = ctx.enter_context(tc.tile_pool(name=R + "escr", bufs=4))
        psum_sim = ctx.enter_context(
            tc.tile_pool(name=R + "psum_sim", bufs=2, space="PSUM")
        )
        build_ctx = ExitStack()
        probp = build_ctx.enter_context(tc.tile_pool(name=R + "probp", bufs=1))
        psum_b = build_ctx.enter_context(
            tc.tile_pool(name=R + "psum_b", bufs=2, space="PSUM")
        )
        dram = ctx.enter_context(tc.tile_pool(name=R + "dram", bufs=1, space="DRAM"))

        # ---- constants -------------------------------------------------
        bigc_sb = const.tile([128, BC_COLS], F32, tag="bigc")
        wfin_sb = const.tile([1, NRB], F32, tag="wfin")
        prob_sb = probp.tile([128, C, 128], F32, tag="prob")
        # group-0 prob slice ships first: it heads the critical path
        g1_0 = GROUPS[0][1]
        nc.sync.dma_start(prob_sb[:, 0:g1_0, :], probw.ap()[:, 0 : 128 * g1_0])
        nc.sync.dma_start(bigc_sb[:], bigc.ap())
        nc.sync.dma_start(wfin_sb[:], wfin.ap())
        lexcl_sb = bigc_sb[:, BC_LEXCL : BC_LEXCL + 128]
        ident_sb = bigc_sb[:, BC_IDENT : BC_IDENT + 128]
        rankb_sb = bigc_sb[:, BC_RANKB : BC_RANKB + 8 * C]
        poff_sb = bigc_sb[:, BC_POFF : BC_POFF + 1]
        rep16_sb = bigc_sb[0:16, BC_REP16 : BC_REP16 + 128]
        oofs_sb = bigc_sb[:, BC_OOFS : BC_OOFS + 800]
        psel_sb = bigc_sb[:, BC_PSEL : BC_PSEL + 2 * NRB]
        ones_p = const.tile([128, 1], F32, tag="ones_p")
        ones_r = const.tile([1, 128], F32, tag="ones_r")
        nc.vector.memset(ones_p[:], 1.0)
        nc.vector.memset(ones_r[:], 1.0)
        # dummy activations preload the Ln/Exp function tables off the
        # critical path (LoadActFuncSet is ~1.3us)
        warm = const.tile([128, 1], F32, tag="warm")
        nc.scalar.activation(warm[:], ones_p[:], AF.Ln)
        nc.scalar.activation(warm[:], ones_p[:], AF.Exp, scale=-0.5)
        nc.scalar.activation(warm[:], ones_p[:], AF.Exp, scale=1.0 / TEMP)

        # ---- build: per-group topk -> indices -> gather ----------------
        if "mini" in variant:
            mini_ps = psum_b.tile([128, 128], F32, tag="sm128")
            nc.tensor.matmul(mini_ps[:], lexcl_sb, prob_sb[:, 0, :])
            mini = main.tile([1, 1], F32, tag="mini")
            nc.vector.tensor_reduce(
                mini[:], mini_ps[0:1, :], axis=mybir.AxisListType.X, op=ALU.add
            )
            nc.sync.dma_start(out[:], mini[:])
            build_ctx.close()
            return

        taus = main.tile([1, 2 * C], F32, tag="taus")
        vals3 = main.tile([128, C, 8], F32, tag="vals3")
        idxs3 = main.tile([128, C, 8], U16, tag="idxs3")
        rank16 = main.tile([128, 8 * C], I16, tag="rank16")
        cand16 = main.tile([128, 8 * C], I16, tag="cand16")
        evr16 = main.tile([128, C * 128], I16, tag="evr16")
        idx_dram = dram.tile([1, C * 128], I16, name="idx_dram")
        idxs_i16 = main.tile([128, 2, 8 * C], I16, tag="idxs")
        g = main.tile([128, 2, C, 128], F32, tag="g")

        def _stop(src_ap):
            stp = main.tile([1, 1], F32, tag="stop")
            nc.vector.tensor_reduce(
                stp[:], src_ap, axis=mybir.AxisListType.X, op=ALU.add
            )
            nc.sync.dma_start(out[:], stp[:])
            build_ctx.close()

        def emit_build_A(gi, s_lo=0, s_hi=None):
            g0, g1 = GROUPS[gi]
            if gi > 0 and s_lo == 0:
                nc.sync.dma_start(
                    prob_sb[:, g0:g1, :], probw.ap()[:, 128 * g0 : 128 * g1]
                )
            # -- stage A: per-partition top-8, then exact threshold over
            # the 1024 candidates (the global top-128 is a subset whp)
            hi = g1 if s_hi is None else min(g0 + s_hi, g1)
            for s in range(g0 + s_lo, hi):
                nc.vector.max(vals3[:, s, :], prob_sb[:, s, :])
                nc.vector.max_index(idxs3[:, s, :], vals3[:, s, :], prob_sb[:, s, :])
            for s in range(g0 + s_lo, hi):
                nc.gpsimd.kth_largest(
                    taus[0:1, 2 * s : 2 * s + 2],
                    vals3[:, s, :],
                    n_per_lane=8,
                    k=K + 2,
                    quantile=QUANT_CAND,
                )

        def emit_build_BD(gi):
            g0, g1 = GROUPS[gi]
            gsz = g1 - g0
            taub_ps = psum_b.tile([128, 512], F32, tag="bsm")
            nc.tensor.matmul(
                taub_ps[:, 0:gsz],
                ones_r[:],
                taus[0:1, 2 * g0 + 1 : 2 * g1 : 2],
            )
            # -- stage B: ranking --------------------------------------
            valid3 = scr.tile([128, 15, 8], F32, tag="valid3")
            nc.vector.tensor_tensor(
                valid3[:, 0:gsz, :],
                vals3[:, g0:g1, :],
                taub_ps[:, 0:gsz]
                .rearrange("p (c o) -> p c o", o=1)
                .to_broadcast([128, gsz, 8]),
                op=ALU.is_gt,
            )
            rowcnt = scr.tile([128, 15], F32, tag="rowcnt")
            nc.vector.tensor_reduce(
                rowcnt[:, 0:gsz], valid3[:, 0:gsz, :],
                axis=mybir.AxisListType.X, op=ALU.add,
            )
            rowoff_ps = psum_b.tile([128, 512], F32, tag="bsm")
            nc.tensor.matmul(rowoff_ps[:, 0:gsz], lexcl_sb, rowcnt[:, 0:gsz])
            ranks = scr.tile([128, 15, 8], F32, tag="ranks")
            nc.vector.tensor_tensor(
                ranks[:, 0:gsz, :],
                rowoff_ps[:, 0:gsz]
                .rearrange("p (c o) -> p c o", o=1)
                .to_broadcast([128, gsz, 8]),
                rankb_sb[:, 8 * g0 : 8 * g1].rearrange("p (c t) -> p c t", t=8),
                op=ALU.add,
            )
            nc.vector.tensor_tensor(
                ranks[:, 0:gsz, :], ranks[:, 0:gsz, :], valid3[:, 0:gsz, :],
                op=ALU.mult,
            )
            nc.vector.tensor_scalar_add(ranks[:, 0:gsz, :], ranks[:, 0:gsz, :], -1.0)
            nc.vector.tensor_copy(
                rank16[:, 8 * g0 : 8 * g1],
                ranks[:, 0:gsz, :].rearrange("p c t -> p (c t)"),
            )
            gidxf = scr.tile([128, 15 * 8], F32, tag="gidxf")
            nc.vector.tensor_scalar(
                gidxf[:, 0 : 8 * gsz],
                idxs3[:, g0:g1, :].rearrange("p c t -> p (c t)"),
                poff_sb, None, op0=ALU.add,
            )
            nc.vector.tensor_copy(cand16[:, 8 * g0 : 8 * g1], gidxf[:, 0 : 8 * gsz])
            # -- compact to batch-index list (gpsimd scatter) ------------
            nc.gpsimd.local_scatter(
                evr16[:, 128 * g0 : 128 * g1],
                cand16[:, 8 * g0 : 8 * g1],
                rank16[:, 8 * g0 : 8 * g1],
                channels=128, num_elems=128 * gsz, num_idxs=8 * gsz,
            )
            # cross-partition collapse, 512-col chunks, psum -> SBUF row
            allidx = main.tile([1, C * 128], I16, tag="allidx")
            nchunk = (128 * gsz + MM_CHUNK - 1) // MM_CHUNK
            for ci in range(nchunk):
                c0 = MM_CHUNK * ci
                c1 = min(MM_CHUNK * (ci + 1), 128 * gsz)
                evrf = gscr.tile([128, 512], F32, tag="evrf")
                nc.vector.tensor_copy(
                    evrf[:, 0 : c1 - c0], evr16[:, 128 * g0 + c0 : 128 * g0 + c1]
                )
                aps = psum_b.tile([128, 512], F32, tag="bsm")
                nc.tensor.matmul(
                    aps[0:1, 0 : c1 - c0], ones_p[:], evrf[:, 0 : c1 - c0]
                )
                nc.vector.tensor_copy(
                    allidx[:, 128 * g0 + c0 : 128 * g0 + c1],
                    aps[0:1, 0 : c1 - c0],
                )
            # -- wrap into 16 partitions (DRAM roundtrip) ---------------
            nc.sync.dma_start(
                idx_dram[:, 128 * g0 : 128 * g1],
                allidx[:, 128 * g0 : 128 * g1],
            )
            wrapped = main.tile([16, 8 * C], I16, tag="wrapped")
            nc.sync.dma_start(
                wrapped[:, 8 * g0 : 8 * g1],
                idx_dram[:, 128 * g0 : 128 * g1].rearrange(
                    "p (s m) -> (p m) s", m=16
                ),
            )
            wrapf = scr.tile([16, 8 * 15], F32, tag="wrapf")
            nc.vector.tensor_copy(wrapf[:, 0 : 8 * gsz], wrapped[:, 8 * g0 : 8 * g1])
            widx_ps = psum_b.tile([128, 512], F32, tag="bsm")
            nc.tensor.matmul(
                widx_ps[:, 0 : 8 * gsz], rep16_sb, wrapf[:, 0 : 8 * gsz]
            )
            for h in range(2):
                nc.vector.tensor_tensor(
                    idxs_i16[:, h, 8 * g0 : 8 * g1],
                    widx_ps[:, 0 : 8 * gsz],
                    oofs_sb[:, 400 * h + 8 * g0 : 400 * h + 8 * g1],
                    op=ALU.add,
                )
        def emit_build_D(gi):
            g0, g1 = GROUPS[gi]
            gsz = g1 - g0
            # -- stage D: gathers (<=1024 idxs each) ---------------------
            for h in range(2):
                for s0, s1 in [(i, min(i + 8, gsz)) for i in range(0, gsz, 8)]:
                    nidx = 128 * (s1 - s0)
                    nc.gpsimd.dma_gather(
                        g[:, h, g0 + s0 : g0 + s1, :],
                        z.ap(),
                        idxs_i16[:, h, 8 * (g0 + s0) : 8 * (g0 + s1)],
                        num_idxs=nidx, num_idxs_reg=nidx, elem_size=D,
                    )

        # ---- stage E: normalize + fused transpose into bf16 table ------
        # one table tile per 2-cluster pack (512 cols, single writer) so a
        # sweep matmul's read depends only on its own pack's transpose
        flats = []
        for q in range(C // 2):
            ft = main.tile(
                [128, 512], BF16, tag=f"flatT{q}", name=f"{R}flatT{q}",
            )
            flats.append(ft)

        def tbl(c0, c1):
            """AP for global table cols [c0, c1) — within one 512-col pack."""
            q, off = c0 // 512, c0 % 512
            assert c1 <= 512 * (q + 1), (c0, c1)
            return flats[q][:, off : off + (c1 - c0)]

        rnw = main.tile([128, C, 2], F32, tag="rnw")

        def emit_build_E(gi):
            g0, g1 = GROUPS[gi]
            gsz = g1 - g0
            sqs = scr.tile([128, 15, 2], F32, tag="sqs")
            for h in range(2):
                sqg = gscr.tile([128, 15, 128], F32, tag="sqg")
                nc.vector.tensor_tensor(
                    sqg[:, 0:gsz, :], g[:, h, g0:g1, :], g[:, h, g0:g1, :],
                    op=ALU.mult,
                )
                nc.vector.tensor_reduce(
                    sqs[:, 0:gsz, h], sqg[:, 0:gsz, :],
                    axis=mybir.AxisListType.X, op=ALU.add,
                )
            lnv = scr.tile([128, 15, 2], F32, tag="lnv")
            nc.scalar.activation(lnv[:, 0:gsz, :], sqs[:, 0:gsz, :], AF.Ln)
            nc.scalar.activation(
                rnw[:, g0:g1, :], lnv[:, 0:gsz, :], AF.Exp, scale=-0.5
            )
            # diag(rnw_block) per 2-cluster pack (4 blocks, h interleaved);
            # one f32 matmul per block fuses normalize+transpose; the pack
            # lands contiguously: flatT[:, 256c .. 256c+512)
            for p0 in range(0, gsz, 2):
                npk = min(2, gsz - p0)
                diag4 = gscr.tile([128, 2, 2, 128], F32, tag="diag4")
                nc.vector.tensor_tensor(
                    diag4[:, 0:npk, :, :],
                    ident_sb.rearrange("p (o u f) -> p o u f", o=1, u=1)
                    .to_broadcast([128, npk, 2, 128]),
                    rnw[:, g0 + p0 : g0 + p0 + npk, :]
                    .rearrange("p c (h o) -> p c h o", o=1)
                    .to_broadcast([128, npk, 2, 128]),
                    op=ALU.mult,
                )
                tp_ps = psum_b.tile([128, 512], F32, tag="bsm")
                for j in range(npk):
                    for h in range(2):
                        nc.tensor.matmul(
                            tp_ps[:, 256 * j + 128 * h : 256 * j + 128 * h + 128],
                            g[:, h, g0 + p0 + j, :],
                            diag4[:, j, h, :],
                        )
                nc.vector.tensor_copy(
                    flats[(g0 + p0) // 2][:, 0 : 256 * npk],
                    tp_ps[:, 0 : 256 * npk],
                )

        partials = main.tile([128, NRB, N_ST], F32, tag="partials")
        eown = main.tile([128, NRB, 256], BF16, tag="eown")

        def emit_sweep_st(st):
            col = ST * st
            stw = ST_SIZES[st]
            for n in range(NRB):
                s, h = SWEEP_SH[n]
                lhsT = tbl(256 * s + 128 * h, 256 * s + 128 * h + 128)
                sim_ps = psum_sim.tile([128, ST], F32, tag="sim")
                for q0 in range(0, stw, MM_CHUNK):
                    mm = nc.tensor.matmul(
                        sim_ps[:, q0 : q0 + MM_CHUNK],
                        lhsT,
                        tbl(col + q0, col + q0 + MM_CHUNK),
                    )
                    if "ldwe" in variant and q0 > 0:
                        mm.ins.ldweights = False
                e_sb = escr.tile([128, ST], BF16, tag="e")
                nc.scalar.activation(
                    e_sb[:, 0:stw],
                    sim_ps[:, 0:stw],
                    AF.Exp,
                    scale=1.0 / TEMP,
                    accum_out=partials[:, n, st : st + 1],
                )
                # own-cluster block: cols [256s, 256s+256) sit entirely in
                # tile 0 (s<6) or at the head of tile 1 (s==6)
                own_st, own_off = (0, 256 * s) if s < 6 else (1, 0)
                if st == own_st:
                    nc.vector.tensor_copy(
                        eown[:, n, :], e_sb[:, own_off : own_off + 256]
                    )

        # interleave emission: per-engine queues are in-order, so sweep
        # tiles must be emitted as soon as their producing groups are
        own_t = main.tile([128, NRB], F32, tag="own_t")
        pos_t = main.tile([128, NRB], F32, tag="pos_t")
        lnp = scr.tile([128, NRB], F32, tag="lnp")

        def emit_own_reduce():
            nc.vector.tensor_reduce(
                own_t[:], eown[:], axis=mybir.AxisListType.X, op=ALU.add
            )
            subs = scr.tile([128, NRB, 2], F32, tag="subs")
            nc.vector.tensor_reduce(
                subs[:],
                eown[:].rearrange("p n (u x) -> p n u x", u=2, x=128),
                axis=mybir.AxisListType.X, op=ALU.add,
            )
            nc.vector.tensor_tensor(
                subs[:],
                subs[:],
                psel_sb.rearrange("p (n u) -> p n u", u=2),
                op=ALU.mult,
            )
            nc.vector.tensor_reduce(
                pos_t[:], subs[:], axis=mybir.AxisListType.X, op=ALU.add
            )
            nc.scalar.activation(lnp[:], pos_t[:], AF.Ln)

        SWEEP_AFTER = [[0], [1, 2], [3, 4, 5], [6, 7, 8]]
        NG = len(GROUPS)
        # software-pipeline the in-order Pool queue: slices of group gi+1's
        # kth_largest batch are emitted into gi's pool-idle windows (after
        # gi's scatter, and between gi's gathers and stage E)
        emit_build_A(0)
        for gi in range(NG):
            emit_build_BD(gi)
            if gi + 1 < NG:
                emit_build_A(gi + 1, 0, 4)
            emit_build_D(gi)
            if gi + 1 < NG:
                emit_build_A(gi + 1, 4)
            if "stopD" in variant:
                continue
            emit_build_E(gi)
            if "stopE" in variant or "nosweep" in variant:
                continue
            for st in SWEEP_AFTER[gi]:
                emit_sweep_st(st)
            if gi == 1:
                emit_own_reduce()
        if "stopD" in variant:
            _stop(g[0:1, 0, 0, :])
            return
        if "stopE" in variant or "nosweep" in variant:
            ef = scr.tile([1, 512], F32, tag="ef")
            nc.vector.tensor_copy(ef[:], flats[0][0:1, :])
            _stop(ef[0:1, :])
            return
        # ---- reduce to one scalar --------------------------------------
        totals = main.tile([128, NRB], F32, tag="totals")
        nc.vector.tensor_reduce(
            totals[:], partials[:], axis=mybir.AxisListType.X, op=ALU.add
        )
        neg = scr.tile([128, NRB], F32, tag="neg")
        nc.vector.tensor_sub(neg[:], totals[:], own_t[:])
        lnn = scr.tile([128, NRB], F32, tag="lnn")
        nc.scalar.activation(lnn[:], neg[:], AF.Ln)
        lrows = main.tile([128, NRB], F32, tag="lrows")
        nc.vector.tensor_sub(lrows[:], lnn[:], lnp[:])
        fin_ps = psum_sim.tile([1, NRB], F32, tag="sim")
        nc.tensor.matmul(fin_ps[:], ones_p[:], lrows[:])
        fin_sb = main.tile([1, NRB], F32, tag="fin_sb")
        nc.vector.tensor_tensor(fin_sb[:], fin_ps[:], wfin_sb[:], op=ALU.mult)
        out_sb = main.tile([1, 1], F32, tag="out_sb")
        nc.vector.tensor_reduce(
            out_sb[:], fin_sb[:], axis=mybir.AxisListType.X, op=ALU.add
        )
        nc.vector.tensor_scalar_mul(out_sb[:], out_sb[:], 1.0 / (2 * K * C))
        nc.sync.dma_start(out[:], out_sb[:])
        build_ctx.close()


# row-block assignment: 46 "full" clusters owned 6/6/6/6/6/6/5/5, plus 4
# "split" clusters whose two z-halves are swept by partner cores (k, k+1).
SPLIT = [46, 47, 48, 49]
FULL_CNT = [6, 6, 6, 6, 6, 6, 5, 5]
FULL_BASE = [0, 6, 12, 18, 24, 30, 36, 41]


def _per_core_inputs(prob, z_i, z_j):
    bigc_base = _host_constants()
    zcat = np.ascontiguousarray(np.concatenate([z_i, z_j], axis=0))
    maps = []
    for k in range(N_CORES):
        own = list(range(FULL_BASE[k], FULL_BASE[k] + FULL_CNT[k]))
        xk = SPLIT[k // 2]
        slots = own.copy()
        if len(slots) == 5:
            filler = FULL_BASE[6] if k == 7 else FULL_BASE[7]
            slots.append(filler)
        slots.append(xk)  # slot 6
        rest = [c for c in range(C) if c not in slots]
        perm = slots + rest
        assert len(perm) == C and len(set(perm)) == C
        pw = np.ascontiguousarray(
            prob[:, perm].T.reshape(C, 128, 128).transpose(1, 0, 2)
            .reshape(128, C * 128)
        )
        swapped = (k % 2 == 1)
        # wfin: weight per swept block
        w = np.zeros(NRB, dtype=np.float32)
        for n, (sl, h) in enumerate(SWEEP_SH):
            if sl < len(own) or sl == 6:
                w[n] = 1.0
        # gather half-offsets O[h, slot] (0 -> z_i rows, B -> z_j rows)
        oofs = np.zeros((2, C), dtype=np.float32)
        oofs[1, :] = float(B)
        if swapped:
            oofs[0, 6], oofs[1, 6] = float(B), 0.0
        # pos selector: which eown sub-block holds the z_i half
        psel = np.zeros((NRB, 2), dtype=np.float32)
        psel[:, 0] = 1.0
        if swapped:
            psel[NRB - 1] = [0.0, 1.0]
        bigc = bigc_base.copy()
        bigc[:, BC_OOFS : BC_OOFS + 800] = np.repeat(
            oofs.reshape(2 * C), 8
        )[None, :]
        bigc[:, BC_PSEL : BC_PSEL + 2 * NRB] = psel.reshape(1, 2 * NRB)
        maps.append({
            "probw": pw,
            "z": zcat,
            "bigc": bigc,
            "wfin": w[None, :].copy(),
        })
    return maps


def kernel(prob, z_i, z_j):
    if "nc" not in _CACHE:
        _CACHE["nc"] = _build_program()
    nc = _CACHE["nc"]
    in_maps = _per_core_inputs(
        np.asarray(prob, dtype=np.float32),
        np.asarray(z_i, dtype=np.float32),
        np.asarray(z_j, dtype=np.float32),
    )
    res = run_bass_kernel_spmd(nc, in_maps, list(range(N_CORES)))
    total = np.float32(0.0)
    for r in res.results:
        total += r["partial0"][0, 0]
    return np.asarray(total, dtype=np.float32)
